# revision 12
# baseline (speedup 1.0000x reference)
"""Trainium2 Bass/Tile kernel for BasicCondConvBlock (E=1):
two CondConv1d(k=3,pad=1)+BN(eval)+LeakyReLU(0.1) blocks + MaxPool1d(2).

With a single expert, CondConv reduces to y_i = r_i * (conv(x_i, W) + b)
with a shared weight: conv runs on the TensorEngine as 3 shifted matmuls
accumulated in PSUM (center tap first at full width; edge taps clipped so
no zero-padding columns are ever materialized), and routing r_i + conv
bias + BatchNorm fold into one per-(sample,channel) affine applied in the
epilogue:
    out = LeakyReLU( (r_i*s_c) * z + (r_i*b_c*s_c + be_c - rm_c*s_c) )
where s_c = g_c / sqrt(rv_c + eps).  LeakyReLU(v) = max(0.1*v, v) is one
fused DVE scalar_tensor_tensor op.  Matmuls run in fp32r (full-rate fp32
on the PE); every producer of a matmul operand writes fp32r-typed tiles.

Sharding: pure data parallel over batch (32 samples -> 4 per core x 8).
"""

import numpy as np

N_CORES = 8
B, CIN, W = 32, 64, 2048
C1, C2 = 128, 256
BL = B // N_CORES  # samples per core
EPS = 1e-5
SLOPE = 0.1
WT = 512           # conv output tile width (one PSUM bank of fp32)
NT = W // WT       # 4
WO = W // 2        # pooled output width
HT = WT // 2

# packed parameter-row offsets
OFF_S1, OFF_T11, OFF_T21 = 0, C1, 2 * C1
OFF_S2, OFF_T12, OFF_T22 = 3 * C1, 3 * C1 + C2, 3 * C1 + 2 * C2
OFF_FCB1 = 3 * C1 + 3 * C2
OFF_FCB2 = OFF_FCB1 + 1
OFF_ONES = OFF_FCB2 + 1
NPROW = OFF_ONES + BL

TRACE = False
LAST_RESULT = None

_built = None


def _build():
    global _built
    if _built is not None:
        return _built

    import concourse.bacc as bacc
    import concourse.mybir as mybir
    from concourse import tile
    from contextlib import ExitStack

    f32 = mybir.dt.float32
    f32r = mybir.dt.float32r
    Alu = mybir.AluOpType
    Act = mybir.ActivationFunctionType

    nc = bacc.Bacc("TRN2", target_bir_lowering=False, debug=False)

    xd = nc.declare_dram_parameter("x", [BL, CIN, W + 2], f32r, isOutput=False)
    w1d = nc.declare_dram_parameter("w1t", [CIN, 3 * C1], f32r, isOutput=False)
    w2d = nc.declare_dram_parameter("w2t", [C1, 3 * C2], f32r, isOutput=False)
    f1d = nc.declare_dram_parameter("fcw1c", [CIN, 1], f32, isOutput=False)
    f2d = nc.declare_dram_parameter("fcw2c", [C1, 1], f32, isOutput=False)
    pd = nc.declare_dram_parameter("prow", [1, NPROW], f32, isOutput=False)
    od = nc.declare_dram_parameter("out", [BL, C2, WO], f32, isOutput=True)
    x_ap, w1_ap, w2_ap = xd.ap(), w1d.ap(), w2d.ap()
    f1_ap, f2_ap, p_ap, o_ap = f1d.ap(), f2d.ap(), pd.ap(), od.ap()

    def conv_taps(zp, lhsT, src, c0):
        """Accumulate the 3-tap conv for output cols [c0, c0+WT) of one
        128-wide output-channel chunk.  lhsT(k) -> [K,128] stationary AP;
        src -> [K, W+2] zero-padded input AP (data at cols 1..W).  All taps
        full width: fp32r matmuls need even N and 8B-aligned PSUM offsets."""
        for k in range(3):
            nc.tensor.matmul(zp[:, 0:WT], lhsT(k), src[:, c0 + k : c0 + k + WT],
                             start=(k == 0), stop=(k == 2))

    with tile.TileContext(nc) as tc:
        with ExitStack() as ctx:
            consts = ctx.enter_context(tc.tile_pool(name="consts", bufs=1))
            xpool = ctx.enter_context(tc.tile_pool(name="xp", bufs=2))
            y1pool = ctx.enter_context(tc.tile_pool(name="y1p", bufs=BL))
            scr = ctx.enter_context(tc.tile_pool(name="scr", bufs=2))
            epi1 = ctx.enter_context(tc.tile_pool(name="epi1", bufs=3))
            epi2w = ctx.enter_context(tc.tile_pool(name="epi2w", bufs=3))
            pmp = ctx.enter_context(tc.tile_pool(name="pmp", bufs=3))
            outp = ctx.enter_context(tc.tile_pool(name="outp", bufs=3))
            small = ctx.enter_context(tc.tile_pool(name="small", bufs=1))
            psum = ctx.enter_context(tc.tile_pool(name="psum", bufs=5, space="PSUM"))
            psmall = ctx.enter_context(tc.tile_pool(name="psm", bufs=1, space="PSUM"))

            sync = nc.sync

            # constants; w1/fcw1 duplicated into partitions 64:128 so matmuls
            # for odd samples (whose data sits at base partition 64) line up
            w1s = consts.tile([2 * CIN, 3 * C1], f32r)
            sync.dma_start(out=w1s[0:CIN, :], in_=w1_ap[:])
            sync.dma_start(out=w1s[CIN : 2 * CIN, :], in_=w1_ap[:])
            w2s = consts.tile([C1, 3 * C2], f32r)
            sync.dma_start(out=w2s[:], in_=w2_ap[:])
            f1s = consts.tile([2 * CIN, 1], f32)
            sync.dma_start(out=f1s[0:CIN, :], in_=f1_ap[:])
            sync.dma_start(out=f1s[CIN : 2 * CIN, :], in_=f1_ap[:])
            f2s = consts.tile([C1, 1], f32)
            sync.dma_start(out=f2s[:], in_=f2_ap[:])
            prs = consts.tile([1, NPROW], f32)
            sync.dma_start(out=prs[:], in_=p_ap[:])

            # x tiles: two samples stacked on the partition dim; zero padding
            # columns come in with the (host-padded) DMA
            xts = []
            for i in range(BL // 2):
                xt = xpool.tile([2 * CIN, W + 2], f32r, tag="xt")
                sync.dma_start(
                    out=xt[:],
                    in_=x_ap[2 * i : 2 * i + 2].rearrange("s c w -> (s c) w"),
                )
                xts.append(xt)

            def xv(s):
                return xts[s // 2][(s % 2) * CIN : (s % 2 + 1) * CIN, :]

            # per-pair column sums of x (ScalarE copy w/ accumulate) -> routing 1.
            # Each scan covers two samples at once (partitions 0:64 / 64:128).
            m1 = small.tile([2 * CIN, BL // 2], f32)
            for i in range(BL // 2):
                sct = scr.tile([2 * CIN, W], f32, tag="sct")
                nc.scalar.activation(
                    sct[:], xts[i][:, 1 : W + 1].bitcast(f32), Act.Copy,
                    accum_out=m1[:, i : i + 1],
                )

            # r1 = sigmoid(fcw1/W . xsum + fcb1) -> [1, BL] in column order
            # [s0, s2, s1, s3] (even samples first, then odd)
            r1 = small.tile([1, BL], f32)
            for half in range(2):
                lg1 = psmall.tile([1, BL // 2], f32, tag="lg")
                nc.tensor.matmul(
                    lg1[:],
                    f1s[half * CIN : (half + 1) * CIN, :],
                    m1[half * CIN : (half + 1) * CIN, :],
                    start=True, stop=True,
                )
                nc.scalar.activation(
                    r1[0:1, half * (BL // 2) : (half + 1) * (BL // 2)],
                    lg1[:], Act.Sigmoid,
                    bias=prs[0:1, OFF_FCB1 : OFF_FCB1 + 1], scale=1.0,
                )

            def colmap(s):
                # column of sample s in r1 / sc1 / bi1 tiles
                return (s % 2) * (BL // 2) + s // 2

            ones = prs[0:1, OFF_ONES : OFF_ONES + BL]

            def outer_pair(s_off, t1_off, t2_off, r_row, cw):
                """scale[c,i] = s_c*r_i ; bias[c,i] = t1_c*r_i + t2_c"""
                opa = psmall.tile([cw, BL], f32, tag="op")
                nc.tensor.matmul(
                    opa[:], prs[0:1, s_off : s_off + cw], r_row[:], start=True, stop=True
                )
                sc = small.tile([cw, BL], f32, tag=f"sc{s_off}")
                nc.scalar.activation(sc[:], opa[:], Act.Copy)
                opb = psmall.tile([cw, BL], f32, tag="op")
                nc.tensor.matmul(
                    opb[:], prs[0:1, t1_off : t1_off + cw], r_row[:], start=True, stop=False
                )
                nc.tensor.matmul(
                    opb[:], prs[0:1, t2_off : t2_off + cw], ones, start=False, stop=True
                )
                bi = small.tile([cw, BL], f32, tag=f"bi{t1_off}")
                nc.scalar.activation(bi[:], opb[:], Act.Copy)
                return sc, bi

            sc1, bi1 = outer_pair(OFF_S1, OFF_T11, OFF_T21, r1, C1)

            # ---- block 1: conv(64->128) + affine + leaky, accumulate row sums
            s1acc = small.tile([C1, BL * NT], f32)
            y1s = []
            for s in range(BL):
                y1 = y1pool.tile([C1, W + 2], f32r, tag="y1")
                # zero the two padding columns with an fp32r-writing DVE op
                # (memset cannot emit fp32r); inputs only feed a *0.0
                nc.vector.scalar_tensor_tensor(
                    y1[:, 0 : W + 2 : W + 1],
                    sc1[:, 0:2], 0.0, sc1[:, 0:2], Alu.mult, Alu.mult,
                )
                half = s % 2
                col = colmap(s)
                w1v = lambda k, h=half: w1s[
                    h * CIN : (h + 1) * CIN, k * C1 : (k + 1) * C1
                ]
                for t in range(NT):
                    zp = psum.tile([C1, WT], f32, tag="zp")
                    conv_taps(zp, w1v, xv(s), WT * t)
                    yt = epi1.tile([C1, WT], f32, tag="yt")
                    nc.scalar.activation(
                        yt[:], zp[:], Act.Identity,
                        bias=bi1[:, col : col + 1], scale=sc1[:, col : col + 1],
                    )
                    nc.vector.scalar_tensor_tensor(
                        y1[:, 1 + WT * t : 1 + WT * (t + 1)],
                        yt[:], SLOPE, yt[:], Alu.mult, Alu.max,
                        accum_out=s1acc[:, NT * s + t : NT * s + t + 1],
                    )
                y1s.append(y1)

            # r2 from block-1 output row sums (natural sample order)
            ta = small.tile([C1, BL], f32)
            tb = small.tile([C1, BL], f32)
            ssum = small.tile([C1, BL], f32)
            nc.vector.tensor_add(ta[:], s1acc[:, 0::NT], s1acc[:, 1::NT])
            nc.vector.tensor_add(tb[:], s1acc[:, 2::NT], s1acc[:, 3::NT])
            nc.vector.tensor_add(ssum[:], ta[:], tb[:])
            lg2 = psmall.tile([1, BL], f32, tag="lg")
            nc.tensor.matmul(lg2[:], f2s[:], ssum[:], start=True, stop=True)
            r2 = small.tile([1, BL], f32)
            nc.scalar.activation(
                r2[:], lg2[:], Act.Sigmoid,
                bias=prs[0:1, OFF_FCB2 : OFF_FCB2 + 1], scale=1.0,
            )
            sc2a, bi2a = outer_pair(OFF_S2, OFF_T12, OFF_T22, r2, C1)
            sc2b, bi2b = outer_pair(OFF_S2 + C1, OFF_T12 + C1, OFF_T22 + C1, r2, C1)
            sc2s, bi2s = (sc2a, sc2b), (bi2a, bi2b)

            # ---- block 2: conv(128->256) + affine + pool + leaky
            for s in range(BL):
                for c in range(2):
                    ot = outp.tile([C1, WO], f32, tag="ot")
                    w2v = lambda k, cc=c: w2s[:, k * C2 + C1 * cc : k * C2 + C1 * cc + C1]
                    for t in range(NT):
                        zp2 = psum.tile([C1, WT], f32, tag="zp")
                        conv_taps(zp2, w2v, y1s[s], WT * t)
                        yt2 = epi2w.tile([C1, WT], f32, tag="yt2")
                        nc.scalar.activation(
                            yt2[:], zp2[:], Act.Identity,
                            bias=bi2s[c][:, s : s + 1], scale=sc2s[c][:, s : s + 1],
                        )
                        pm = pmp.tile([C1, HT], f32, tag="pm")
                        nc.vector.tensor_tensor(
                            pm[:], yt2[:, 0:WT:2], yt2[:, 1:WT:2], Alu.max
                        )
                        nc.vector.scalar_tensor_tensor(
                            ot[:, HT * t : HT * (t + 1)],
                            pm[:], SLOPE, pm[:], Alu.mult, Alu.max,
                        )
                    sync.dma_start(
                        out=o_ap[s, C1 * c : C1 * (c + 1), :], in_=ot[:]
                    )

    nc.compile()
    _built = nc
    return nc


def _pack_inputs(x, w1, b1, fcw1, fcb1, g1, be1, rm1, rv1,
                 w2, b2, fcw2, fcb2, g2, be2, rm2, rv2):
    f = np.float32
    s1 = (g1 / np.sqrt(rv1 + EPS)).astype(f)
    s2 = (g2 / np.sqrt(rv2 + EPS)).astype(f)
    prow = np.zeros(NPROW, f)
    prow[OFF_S1:OFF_S1 + C1] = s1
    prow[OFF_T11:OFF_T11 + C1] = b1[0] * s1
    prow[OFF_T21:OFF_T21 + C1] = be1 - rm1 * s1
    prow[OFF_S2:OFF_S2 + C2] = s2
    prow[OFF_T12:OFF_T12 + C2] = b2[0] * s2
    prow[OFF_T22:OFF_T22 + C2] = be2 - rm2 * s2
    prow[OFF_FCB1] = fcb1[0]
    prow[OFF_FCB2] = fcb2[0]
    prow[OFF_ONES:OFF_ONES + BL] = 1.0

    com = {
        "w1t": np.ascontiguousarray(w1[0].transpose(1, 2, 0).reshape(CIN, 3 * C1), f),
        "w2t": np.ascontiguousarray(w2[0].transpose(1, 2, 0).reshape(C1, 3 * C2), f),
        "fcw1c": np.ascontiguousarray((fcw1[0] / W).reshape(CIN, 1), f),
        "fcw2c": np.ascontiguousarray((fcw2[0] / W).reshape(C1, 1), f),
        "prow": prow.reshape(1, NPROW),
    }
    xp = np.zeros((B, CIN, W + 2), f)
    xp[:, :, 1 : W + 1] = x
    return [
        {**com, "x": np.ascontiguousarray(xp[i * BL : (i + 1) * BL])}
        for i in range(N_CORES)
    ]


def _enable_trace():
    """Register the NTFF profile hook (absent antenv.axon_hooks on this image)
    and stub out the S3 artifact upload so trace=True works locally."""
    import sys
    import types

    import concourse.bass_utils as bu

    bu.upload_artifacts = lambda tmpdir: tmpdir
    if "antenv.axon_hooks" not in sys.modules:
        import antenv
        from trn_agent_boot.trn_boot import _ntff_profile_via_ctypes

        hooks = types.ModuleType("antenv.axon_hooks")
        _store = {"hook": _ntff_profile_via_ctypes("/opt/axon/libaxon_pjrt.so")}
        hooks.set_axon_ntff_profile_hook = lambda h: _store.__setitem__("hook", h)
        hooks.get_axon_ntff_profile_hook = lambda: _store["hook"]
        sys.modules["antenv.axon_hooks"] = hooks
        antenv.axon_hooks = hooks


def kernel(**inputs):
    global LAST_RESULT
    from concourse.bass_utils import run_bass_kernel_spmd

    if TRACE:
        _enable_trace()
    nc = _build()
    in_maps = _pack_inputs(**inputs)
    res = run_bass_kernel_spmd(nc, in_maps, list(range(N_CORES)), trace=TRACE)
    LAST_RESULT = res
    return np.concatenate([r["out"] for r in res.results], axis=0)


# revision 14
# speedup vs baseline: 1.0660x; 1.0660x over previous
"""Trainium2 Bass/Tile kernel for BasicCondConvBlock (E=1):
two CondConv1d(k=3,pad=1)+BN(eval)+LeakyReLU(0.1) blocks + MaxPool1d(2).

With a single expert, CondConv reduces to y_i = r_i * (conv(x_i, W) + b)
with a shared weight: conv runs on the TensorEngine as 3 shifted fp32r
matmuls accumulated in PSUM, and routing r_i + conv bias + BatchNorm fold
into one per-(sample,channel) affine.  The whole block-1 epilogue is a
single ScalarE op per tile: Prelu(z*scale + bias, alpha=0.1) (= LeakyReLU)
writing fp32r with a fused row-sum (feeds block-2 routing).  Block 2 pools
first — max over adjacent pairs straight out of PSUM via a one-input
3D-AP tensor_reduce (legal since scale>0 keeps the affine+Prelu monotone)
— then applies Prelu(affine) on the half-width result.

Sharding: pure data parallel over batch (32 samples -> 4 per core x 8).
"""

import numpy as np

N_CORES = 8
B, CIN, W = 32, 64, 2048
C1, C2 = 128, 256
BL = B // N_CORES  # samples per core
EPS = 1e-5
SLOPE = 0.1
WT = 512           # conv output tile width (one PSUM bank of fp32)
NT = W // WT       # 4
WO = W // 2        # pooled output width
HT = WT // 2

# packed parameter-row offsets
OFF_S1, OFF_T11, OFF_T21 = 0, C1, 2 * C1
OFF_S2, OFF_T12, OFF_T22 = 3 * C1, 3 * C1 + C2, 3 * C1 + 2 * C2
OFF_FCB1 = 3 * C1 + 3 * C2
OFF_FCB2 = OFF_FCB1 + 1
OFF_ONES = OFF_FCB2 + 1
NPROW = OFF_ONES + BL

TRACE = False
LAST_RESULT = None

_built = None


def _build():
    global _built
    if _built is not None:
        return _built

    import concourse.bacc as bacc
    import concourse.mybir as mybir
    from concourse import tile
    from contextlib import ExitStack

    f32 = mybir.dt.float32
    f32r = mybir.dt.float32r
    Alu = mybir.AluOpType
    Act = mybir.ActivationFunctionType
    Ax = mybir.AxisListType

    nc = bacc.Bacc("TRN2", target_bir_lowering=False, debug=False)

    xd = nc.declare_dram_parameter("x", [BL, CIN, W + 2], f32r, isOutput=False)
    w1d = nc.declare_dram_parameter("w1t", [CIN, 3 * C1], f32r, isOutput=False)
    w2d = nc.declare_dram_parameter("w2t", [C1, 3 * C2], f32r, isOutput=False)
    f1d = nc.declare_dram_parameter("fcw1c", [CIN, 1], f32, isOutput=False)
    f2d = nc.declare_dram_parameter("fcw2c", [C1, 1], f32, isOutput=False)
    pd = nc.declare_dram_parameter("prow", [1, NPROW], f32, isOutput=False)
    od = nc.declare_dram_parameter("out", [BL, C2, WO], f32, isOutput=True)
    x_ap, w1_ap, w2_ap = xd.ap(), w1d.ap(), w2d.ap()
    f1_ap, f2_ap, p_ap, o_ap = f1d.ap(), f2d.ap(), pd.ap(), od.ap()

    def conv_taps(zp, lhsT, src, c0):
        """Accumulate the 3-tap conv for output cols [c0, c0+WT) of one
        128-wide output-channel chunk.  lhsT(k) -> [K,128] stationary AP;
        src -> [K, W+2] zero-padded input AP (data at cols 1..W).  All taps
        full width: fp32r matmuls need even N and 8B-aligned PSUM offsets."""
        for k in range(3):
            nc.tensor.matmul(zp[:, 0:WT], lhsT(k), src[:, c0 + k : c0 + k + WT],
                             start=(k == 0), stop=(k == 2))

    with tile.TileContext(nc) as tc:
        with ExitStack() as ctx:
            consts = ctx.enter_context(tc.tile_pool(name="consts", bufs=1))
            xpool = ctx.enter_context(tc.tile_pool(name="xp", bufs=2))
            y1pool = ctx.enter_context(tc.tile_pool(name="y1p", bufs=BL))
            pmp = ctx.enter_context(tc.tile_pool(name="pmp", bufs=8))
            outp = ctx.enter_context(tc.tile_pool(name="outp", bufs=3))
            small = ctx.enter_context(tc.tile_pool(name="small", bufs=1))
            psum = ctx.enter_context(tc.tile_pool(name="psum", bufs=6, space="PSUM"))
            psmall = ctx.enter_context(tc.tile_pool(name="psm", bufs=1, space="PSUM"))

            sync = nc.sync

            # --- input DMAs, startup-critical first -------------------------
            # w1 weights (duplicated into partitions 64:128 so matmuls for odd
            # samples, whose data sits at base partition 64, line up)
            w1s = consts.tile([2 * CIN, 3 * C1], f32r)
            sync.dma_start(out=w1s[0:CIN, :], in_=w1_ap[:])
            sync.dma_start(out=w1s[CIN : 2 * CIN, :], in_=w1_ap[:])

            # x tiles: two samples stacked on the partition dim, zero padding
            # columns included by the host-side pad; chunked so the first conv
            # matmuls can start before the whole batch has landed
            XC = [(0, WT + 2), (WT + 2, WT), (2 * WT + 2, WT), (3 * WT + 2, WT)]
            xts = [
                xpool.tile([2 * CIN, W + 2], f32r, tag="xt", name=f"xt{i}")
                for i in range(BL // 2)
            ]
            for c0, cn in XC:
                for i in range(BL // 2):
                    sync.dma_start(
                        out=xts[i][:, c0 : c0 + cn],
                        in_=x_ap[2 * i : 2 * i + 2, :, c0 : c0 + cn].rearrange(
                            "s c w -> (s c) w"
                        ),
                    )

            f1s = consts.tile([2 * CIN, 1], f32)
            sync.dma_start(out=f1s[0:CIN, :], in_=f1_ap[:])
            sync.dma_start(out=f1s[CIN : 2 * CIN, :], in_=f1_ap[:])
            prs = consts.tile([1, NPROW], f32)
            sync.dma_start(out=prs[:], in_=p_ap[:])
            w2s = consts.tile([C1, 3 * C2], f32r)
            sync.dma_start(out=w2s[:], in_=w2_ap[:])
            f2s = consts.tile([C1, 1], f32)
            sync.dma_start(out=f2s[:], in_=f2_ap[:])

            def xv(s):
                return xts[s // 2][(s % 2) * CIN : (s % 2 + 1) * CIN, :]

            # per-pair column sums of x (DVE reduce; covers two samples at
            # once on partitions 0:64 / 64:128) -> routing 1
            m1 = small.tile([2 * CIN, BL // 2], f32)
            for i in range(BL // 2):
                nc.vector.reduce_sum(
                    m1[:, i : i + 1], xts[i][:, 1 : W + 1].bitcast(f32), axis=Ax.X
                )

            # r1 = sigmoid(fcw1/W . xsum + fcb1) -> [1, BL] in column order
            # [s0, s2, s1, s3] (even samples first, then odd)
            r1 = small.tile([1, BL], f32)
            for half in range(2):
                lg1 = psmall.tile([1, BL // 2], f32, tag="lg")
                nc.tensor.matmul(
                    lg1[:],
                    f1s[half * CIN : (half + 1) * CIN, :],
                    m1[half * CIN : (half + 1) * CIN, :],
                    start=True, stop=True,
                )
                nc.scalar.activation(
                    r1[0:1, half * (BL // 2) : (half + 1) * (BL // 2)],
                    lg1[:], Act.Sigmoid,
                    bias=prs[0:1, OFF_FCB1 : OFF_FCB1 + 1], scale=1.0,
                )

            def colmap(s):
                # column of sample s in r1 / sc1 / bi1 tiles
                return (s % 2) * (BL // 2) + s // 2

            ones = prs[0:1, OFF_ONES : OFF_ONES + BL]

            def outer_pair(s_off, t1_off, t2_off, r_row, cw):
                """scale[c,i] = s_c*r_i ; bias[c,i] = t1_c*r_i + t2_c"""
                opa = psmall.tile([cw, BL], f32, tag="op")
                nc.tensor.matmul(
                    opa[:], prs[0:1, s_off : s_off + cw], r_row[:], start=True, stop=True
                )
                sc = small.tile([cw, BL], f32, tag=f"sc{s_off}")
                nc.scalar.activation(sc[:], opa[:], Act.Copy)
                opb = psmall.tile([cw, BL], f32, tag="op")
                nc.tensor.matmul(
                    opb[:], prs[0:1, t1_off : t1_off + cw], r_row[:], start=True, stop=False
                )
                nc.tensor.matmul(
                    opb[:], prs[0:1, t2_off : t2_off + cw], ones, start=False, stop=True
                )
                bi = small.tile([cw, BL], f32, tag=f"bi{t1_off}")
                nc.scalar.activation(bi[:], opb[:], Act.Copy)
                return sc, bi

            sc1, bi1 = outer_pair(OFF_S1, OFF_T11, OFF_T21, r1, C1)

            # ---- block 1: conv(64->128); epilogue = one ScalarE op per tile:
            # Prelu(z*scale + bias, alpha) -> fp32r y1, with fused row-sum
            s1acc = small.tile([C1, BL * NT], f32)
            y1s = []
            for s in range(BL):
                y1 = y1pool.tile([C1, W + 2], f32r, tag="y1")
                # zero the two padding columns with an fp32r-writing DVE op
                # (memset cannot emit fp32r); inputs only feed a *0.0
                nc.vector.scalar_tensor_tensor(
                    y1[:, 0 : W + 2 : W + 1],
                    sc1[:, 0:2], 0.0, sc1[:, 0:2], Alu.mult, Alu.mult,
                )
                half = s % 2
                col = colmap(s)
                w1v = lambda k, h=half: w1s[
                    h * CIN : (h + 1) * CIN, k * C1 : (k + 1) * C1
                ]
                for t in range(NT):
                    zp = psum.tile([C1, WT], f32, tag="zp")
                    conv_taps(zp, w1v, xv(s), WT * t)
                    nc.scalar.activation(
                        y1[:, 1 + WT * t : 1 + WT * (t + 1)], zp[:], Act.Prelu,
                        bias=bi1[:, col : col + 1], scale=sc1[:, col : col + 1],
                        alpha=SLOPE,
                        accum_out=s1acc[:, NT * s + t : NT * s + t + 1],
                    )
                y1s.append(y1)

            # r2 from block-1 output row sums (natural sample order)
            ta = small.tile([C1, BL], f32)
            tb = small.tile([C1, BL], f32)
            ssum = small.tile([C1, BL], f32)
            nc.vector.tensor_add(ta[:], s1acc[:, 0::NT], s1acc[:, 1::NT])
            nc.vector.tensor_add(tb[:], s1acc[:, 2::NT], s1acc[:, 3::NT])
            nc.vector.tensor_add(ssum[:], ta[:], tb[:])
            lg2 = psmall.tile([1, BL], f32, tag="lg")
            nc.tensor.matmul(lg2[:], f2s[:], ssum[:], start=True, stop=True)
            r2 = small.tile([1, BL], f32)
            nc.scalar.activation(
                r2[:], lg2[:], Act.Sigmoid,
                bias=prs[0:1, OFF_FCB2 : OFF_FCB2 + 1], scale=1.0,
            )
            sc2a, bi2a = outer_pair(OFF_S2, OFF_T12, OFF_T22, r2, C1)
            sc2b, bi2b = outer_pair(OFF_S2 + C1, OFF_T12 + C1, OFF_T22 + C1, r2, C1)
            sc2s, bi2s = (sc2a, sc2b), (bi2a, bi2b)

            # ---- block 2: conv(128->256); pool adjacent pairs straight from
            # PSUM (one-input 3D-AP max reduce), then Prelu(affine) at half
            # width.  Pool-before-affine is exact because scale>0.
            for s in range(BL):
                for c in range(2):
                    ot = outp.tile([C1, WO], f32, tag="ot")
                    w2v = lambda k, cc=c: w2s[:, k * C2 + C1 * cc : k * C2 + C1 * cc + C1]
                    for t in range(NT):
                        zp2 = psum.tile([C1, WT], f32, tag="zp")
                        conv_taps(zp2, w2v, y1s[s], WT * t)
                        pm = pmp.tile([C1, HT], f32, tag="pm")
                        nc.vector.tensor_reduce(
                            pm[:], zp2[:].rearrange("p (a b) -> p a b", b=2),
                            axis=Ax.X, op=Alu.max,
                        )
                        nc.scalar.activation(
                            ot[:, HT * t : HT * (t + 1)], pm[:], Act.Prelu,
                            bias=bi2s[c][:, s : s + 1], scale=sc2s[c][:, s : s + 1],
                            alpha=SLOPE,
                        )
                    sync.dma_start(
                        out=o_ap[s, C1 * c : C1 * (c + 1), :], in_=ot[:]
                    )

    nc.compile()
    _built = nc
    return nc


def _pack_inputs(x, w1, b1, fcw1, fcb1, g1, be1, rm1, rv1,
                 w2, b2, fcw2, fcb2, g2, be2, rm2, rv2):
    f = np.float32
    s1 = (g1 / np.sqrt(rv1 + EPS)).astype(f)
    s2 = (g2 / np.sqrt(rv2 + EPS)).astype(f)
    prow = np.zeros(NPROW, f)
    prow[OFF_S1:OFF_S1 + C1] = s1
    prow[OFF_T11:OFF_T11 + C1] = b1[0] * s1
    prow[OFF_T21:OFF_T21 + C1] = be1 - rm1 * s1
    prow[OFF_S2:OFF_S2 + C2] = s2
    prow[OFF_T12:OFF_T12 + C2] = b2[0] * s2
    prow[OFF_T22:OFF_T22 + C2] = be2 - rm2 * s2
    prow[OFF_FCB1] = fcb1[0]
    prow[OFF_FCB2] = fcb2[0]
    prow[OFF_ONES:OFF_ONES + BL] = 1.0

    com = {
        "w1t": np.ascontiguousarray(w1[0].transpose(1, 2, 0).reshape(CIN, 3 * C1), f),
        "w2t": np.ascontiguousarray(w2[0].transpose(1, 2, 0).reshape(C1, 3 * C2), f),
        "fcw1c": np.ascontiguousarray((fcw1[0] / W).reshape(CIN, 1), f),
        "fcw2c": np.ascontiguousarray((fcw2[0] / W).reshape(C1, 1), f),
        "prow": prow.reshape(1, NPROW),
    }
    xp = np.zeros((B, CIN, W + 2), f)
    xp[:, :, 1 : W + 1] = x
    return [
        {**com, "x": np.ascontiguousarray(xp[i * BL : (i + 1) * BL])}
        for i in range(N_CORES)
    ]


def _enable_trace():
    """Register the NTFF profile hook (absent antenv.axon_hooks on this image)
    and stub out the S3 artifact upload so trace=True works locally."""
    import sys
    import types

    import concourse.bass_utils as bu

    bu.upload_artifacts = lambda tmpdir: tmpdir
    if "antenv.axon_hooks" not in sys.modules:
        import antenv
        from trn_agent_boot.trn_boot import _ntff_profile_via_ctypes

        hooks = types.ModuleType("antenv.axon_hooks")
        _store = {"hook": _ntff_profile_via_ctypes("/opt/axon/libaxon_pjrt.so")}
        hooks.set_axon_ntff_profile_hook = lambda h: _store.__setitem__("hook", h)
        hooks.get_axon_ntff_profile_hook = lambda: _store["hook"]
        sys.modules["antenv.axon_hooks"] = hooks
        antenv.axon_hooks = hooks


def kernel(**inputs):
    global LAST_RESULT
    from concourse.bass_utils import run_bass_kernel_spmd

    if TRACE:
        _enable_trace()
    nc = _build()
    in_maps = _pack_inputs(**inputs)
    res = run_bass_kernel_spmd(nc, in_maps, list(range(N_CORES)), trace=TRACE)
    LAST_RESULT = res
    return np.concatenate([r["out"] for r in res.results], axis=0)
